# revision 39
# baseline (speedup 1.0000x reference)
"""Category-specific linear (MoE-style routed batched matmul) on 8 trn2 cores.

out[b, s, h] = sum_i x[b, s, i] * W[cat_ids[b], i, h] + bias[cat_ids[b], h]

Shapes (hardcoded): x (32, 512, 1024) f32, cat_ids (32,) int, W (16, 1024, 4096)
f32, b (16, 4096) f32 -> out (32, 512, 4096) f32.

Strategy: data-parallel over batch, 4 batches per core, with host-side routing
that always packs one same-category PAIR of batches plus two singles per core
(slot capacities [2, 1, 1] batches). With 32 batches over 16 categories there
are always >= (32 - 16)/2 = 8 disjoint same-category pairs, so this packing is
feasible for ANY cat_ids. Each core then loads only 3 weight matrices (24 MB
fp16), keeping DMA (~45 MB/core with fp16 output) under the PE floor
(1024 N=512 matmuls ~= 221 us back-to-back at 78.6 TF/s bf16).

Device kernel = pure matmul; the +bias epilogue runs on host during the f32
upcast (replicating 2048 bias floats across 128 SBUF partitions by DMA was
measured to hog ~250 GB/s of fabric for ~10 us exactly when the opening W
stream was critical, and pacing it proved unreliable).

Layout (from trace analysis):
  - Opening (slot A, half 0) runs nh-chunk-major: W arrives as [128, 1024]
    per-kt chunks interleaved on the sync ring while x^T batches ride the
    scalar ring, so the first matmuls issue ~10 us in and trickle-feed the
    PE while the first 4 MB W half streams.
  - A 16-matmul fp16 warmup flips the HAM clock gate (1.2 -> 2.4 GHz) early
    and bridges the DMA spin-up.
  - All other halves run m-major with whole-half W tiles (one 4 MB DMA
    each), double-buffered; psum as [128, 1024] 2-bank tiles with two
    accumulation groups per tile (zero regions are per-bank).
  - Output is stored fp16 (halves store traffic; host upcasts; ~5e-4 total
    relative error against a 2e-2 budget).
"""

import numpy as np

import concourse.bacc as bacc
import concourse.mybir as mybir
import concourse.bass as bass
import concourse.tile as tile
from concourse.bass_utils import run_bass_kernel_spmd

N_CORES = 8
B, S, K, H = 32, 512, 1024, 4096
BPC = B // N_CORES          # batches per core
P = 128                     # partitions
KT = K // P                 # k tiles (8)
MT = S // P                 # sample tiles per batch (4)
NHALF = 2                   # n halves per weight matrix
NH = H // NHALF             # cols per half (2048)
NH2 = NH // 2               # opening chunk width (1024)
SLOT_BATCHES = (2, 1, 1)    # batches per weight slot
NSLOT = len(SLOT_BATCHES)

_COMPILED = None


def _build():
    nc = bacc.Bacc("TRN2", target_bir_lowering=False, debug=False)
    f32 = mybir.dt.float32
    f16 = mybir.dt.float16

    xt_ap = nc.dram_tensor("xt", [BPC, K, S], f16, kind="ExternalInput").ap()
    # W host-relaid as [slot, half, p, kt, n]: each main-half DMA reads 32 KB
    # contiguous per partition (measured 420-490 GB/s vs ~350 for 4 KB lines);
    # opening chunks slice the same layout with 2 KB lines as before.
    w_ap = nc.dram_tensor(
        "w", [NSLOT, NHALF, P, KT, NH], f16, kind="ExternalInput"
    ).ap()
    out_ap = nc.dram_tensor("out", [BPC, S, H], f16, kind="ExternalOutput").ap()

    with tile.TileContext(nc) as tc:
        with (
            tc.tile_pool(name="warm_pool", bufs=1) as warm_pool,
            tc.tile_pool(name="xtc_pool", bufs=8) as xtc_pool,    # b0 per-kt chunks
            tc.tile_pool(name="xt_pool", bufs=3) as xt_pool,      # whole xt b1..b3
            tc.tile_pool(name="wo_pool", bufs=16) as wo_pool,     # opening W chunks
            tc.tile_pool(name="wm_pool", bufs=2) as wm_pool,      # main W halves
            tc.tile_pool(name="outo_pool", bufs=4) as outo_pool,
            tc.tile_pool(name="outm_pool", bufs=4) as outm_pool,
            tc.tile_pool(name="ps_pool", bufs=4, space="PSUM") as ps_pool,
        ):
            xt_r = [xt_ap[b].rearrange("(kt p) m -> p kt m", p=P) for b in range(BPC)]

            # ---- warmup: flip the HAM clock gate while the first DMAs land.
            warm_x = warm_pool.tile([P, P], f16, name="warm_x", tag="warmx")
            warm_w = warm_pool.tile([P, 512], f16, tag="warmw", name="warm_w")
            nc.vector.memset(warm_x[:], 0.0)
            nc.vector.memset(warm_w[:], 0.0)
            warm_ps = ps_pool.tile([P, NH2], f32, tag="ps", name="warm_ps")
            for _ in range(16):
                nc.tensor.matmul(
                    warm_ps[:, 0:512], warm_x[:], warm_w[:], start=True, stop=True,
                    skip_group_check=True,
                )
            warm_out = warm_pool.tile([P, 4], f32, name="warm_out", tag="warmo")
            nc.vector.tensor_copy(warm_out[:], warm_ps[:, 0:4])

            # ---- opening DMAs: x^T batch 0 as per-kt chunks + batches 1-3
            # whole on the scalar ring; W (slot A half 0) as per-kt [P, 1024]
            # chunks on the sync ring, nh-major, so arrivals match the
            # opening compute order.
            xtc = []
            for kt in range(KT):
                xc = xtc_pool.tile([P, S], f16, name="xc", tag="xtc")
                nc.scalar.dma_start(xc[:], xt_r[0][:, kt, :])
                xtc.append(xc)
            xt_ts = [None]
            for b in range(1, BPC):
                xt = xt_pool.tile([P, KT, S], f16, name="xt_t", tag="xt")
                nc.scalar.dma_start(xt[:], xt_r[b])
                xt_ts.append(xt)
            wo = [[], []]     # [nh][kt] -> [P, NH2]
            for nh in range(2):
                for kt in range(KT):
                    wt = wo_pool.tile([P, NH2], f16, tag="wo", name="wo_t")
                    nc.sync.dma_start(
                        wt[:], w_ap[0, 0, :, kt, nh * NH2 : (nh + 1) * NH2]
                    )
                    wo[nh].append(wt)

            def lhsT(b, kt, m):
                if b == 0:
                    return xtc[kt][:, m * P : (m + 1) * P]
                return xt_ts[b][:, kt, m * P : (m + 1) * P]

            # ---- opening compute: slot A half 0, nh-major, kt-outer so each
            # arriving chunk unlocks matmuls for all 4 m-tiles.
            for nh in range(2):
                for b in range(2):
                    ps = [
                        ps_pool.tile([P, NH2], f32, tag="ps", name="ps")
                        for _ in range(MT)
                    ]
                    for kt in range(KT):
                        for m in range(MT):
                            lt = lhsT(b, kt, m)
                            for n2 in range(2):
                                nc.tensor.matmul(
                                    ps[m][:, n2 * 512 : (n2 + 1) * 512],
                                    lt,
                                    wo[nh][kt][:, n2 * 512 : (n2 + 1) * 512],
                                    start=(kt == 0),
                                    stop=(kt == KT - 1),
                                )
                    for m in range(MT):
                        out_t = outo_pool.tile([P, NH2], f16, tag="outo")
                        nc.vector.tensor_copy(out_t[:], ps[m][:])
                        nc.scalar.dma_start(
                            out_ap[b, m * P : (m + 1) * P, nh * NH2 : (nh + 1) * NH2],
                            out_t[:],
                        )

            # ---- main phase: m-major, whole-half W tiles, double-buffered.
            slot_first_batch = (0, 2, 3)
            main_halves = [(0, 1), (1, 0), (1, 1), (2, 0), (2, 1)]
            for s, half in main_halves:
                nb = SLOT_BATCHES[s]
                bi0 = slot_first_batch[s]
                w_t = wm_pool.tile([P, KT, NH], f16, tag="wm", name="w_t")
                nc.sync.dma_start(w_t[:], w_ap[s, half])
                n_mt = nb * MT
                for ml in range(n_mt):
                    b, mi = divmod(ml, MT)
                    last_tile = (s, half, ml) == (2, 1, n_mt - 1)
                    ps0 = ps_pool.tile([P, NH2], f32, tag="ps", name="ps0")
                    ps1 = ps_pool.tile([P, NH2], f32, tag="ps", name="ps1")
                    pss = (ps0, ps0, ps1, ps1)
                    for kt in range(KT):
                        lt = lhsT(bi0 + b, kt, mi)
                        for n4 in range(4):
                            nc.tensor.matmul(
                                pss[n4][:, (n4 % 2) * 512 : (n4 % 2 + 1) * 512],
                                lt,
                                w_t[:, kt, n4 * 512 : (n4 + 1) * 512],
                                start=(kt == 0),
                                stop=(kt == KT - 1),
                            )
                    dst = out_ap[
                        bi0 + b, mi * P : (mi + 1) * P, half * NH : (half + 1) * NH
                    ]
                    if last_tile:
                        # finer eviction pipeline to shorten the kernel tail
                        for nh in range(2):
                            out_t = outo_pool.tile([P, NH2], f16, tag="outo")
                            nc.vector.tensor_copy(out_t[:], pss[2 * nh][:])
                            nc.scalar.dma_start(
                                dst[:, nh * NH2 : (nh + 1) * NH2], out_t[:]
                            )
                    else:
                        out_t = outm_pool.tile([P, NH], f16, tag="outm")
                        nc.vector.tensor_copy(out_t[:, 0:NH2], ps0[:])
                        nc.vector.tensor_copy(out_t[:, NH2:NH], ps1[:])
                        nc.scalar.dma_start(dst, out_t[:])
    nc.compile()
    return nc


def _get_compiled():
    global _COMPILED
    if _COMPILED is None:
        _COMPILED = _build()
    return _COMPILED


def _pack(cat_ids):
    """Assign batches to cores with slot capacities [2,1,1] per core.

    Returns per-core (idx, slot_cats): idx = 4 batch indices ordered
    [pair0, pair1, single_b, single_c]; slot_cats = categories for the 3 slots.
    Always feasible: #disjoint same-cat pairs = (32 - #odd-count cats)/2 >= 8.
    """
    cat_ids = np.asarray(cat_ids)
    by_cat = {}
    for i, c in enumerate(cat_ids.tolist()):
        by_cat.setdefault(c, []).append(i)
    pairs = []
    singles = []
    for c, idxs in sorted(by_cat.items()):
        n = len(idxs)
        for j in range(n // 2):
            pairs.append((c, idxs[2 * j], idxs[2 * j + 1]))
        if n % 2:
            singles.append((c, idxs[-1]))
    assert len(pairs) >= N_CORES, "impossible: <8 same-cat pairs among 32 batches"
    core_pairs = pairs[:N_CORES]
    # leftovers: extra pairs flatten into singles
    for c, i, j in pairs[N_CORES:]:
        singles.append((c, i))
        singles.append((c, j))
    assert len(singles) == 2 * N_CORES
    cores = []
    for ci in range(N_CORES):
        c, i, j = core_pairs[ci]
        (cb, ib), (cc, ic) = singles[2 * ci], singles[2 * ci + 1]
        cores.append(([i, j, ib, ic], [c, cb, cc]))
    return cores


def run_sharded(x, cat_ids, W, b, trace=False, **spmd_kwargs):
    """Shard, run on 8 cores, unshard (+bias on host). Returns (out, res)."""
    x = np.ascontiguousarray(np.asarray(x), dtype=np.float32)
    cat_ids = np.asarray(cat_ids).astype(np.int64)
    W = np.ascontiguousarray(np.asarray(W), dtype=np.float32)
    b = np.ascontiguousarray(np.asarray(b), dtype=np.float32)

    nc = _get_compiled()
    cores = _pack(cat_ids)

    in_maps = []
    for idx, slot_cats in cores:
        in_maps.append(
            {
                "xt": np.ascontiguousarray(
                    x[idx].transpose(0, 2, 1).astype(np.float16)
                ),
                "w": np.ascontiguousarray(
                    W[slot_cats]
                    .astype(np.float16)
                    .reshape(NSLOT, KT, P, NHALF, NH)
                    .transpose(0, 3, 2, 1, 4)
                ),
            }
        )

    res = run_bass_kernel_spmd(
        nc, in_maps, list(range(N_CORES)), trace=trace, **spmd_kwargs
    )

    out = np.empty((B, S, H), dtype=np.float32)
    for c, (idx, _) in enumerate(cores):
        out[idx] = res.results[c]["out"].astype(np.float32)
    out += b[cat_ids][:, None, :]
    return out, res


def kernel(x, cat_ids, W, b):
    out, _ = run_sharded(x, cat_ids, W, b)
    return out
